# revision 10
# baseline (speedup 1.0000x reference)
"""Trainium2 Bass kernel for a gated bilinear-attention GNN (GAT-with-gate).

Math (per batch b):
    h   = x @ W_w.T + W_b                      [N, D]
    e   = (h A) h^T ; e_sym = e + e^T = h (A + A^T) h^T   (one quadratic form)
    att = softmax(where(adj>0, e_sym, 0), axis=1) * adj
    rv  = h; 3x: az = relu(att @ rv);  c = sigmoid([h, az] @ gate_w.T + gate_b)
               rv = c * h + (1 - c) * az

Device strategy: data-parallel over the batch dim, 2 batches per core on 8
cores.  Layouts:
    attT[j, i] = adj[i, j] * exp(e_sym[j, i]) / denom_j   (pre-normalized!)
    denom_j    = row-sum of masked exp + (N - indeg_j) metadata
    azT[f, i]  = sum_j rv[j, f] * attT[j, i]
    rv_new     = az + c * (h - az)   (combine in natural layout)

v2 changes vs the 97us baseline:
  - adj travels as uint8 (4x less DMA), consumed directly by the DVE/Pool
    mask op.
  - exp-first mask: t = exp(e) on ACT (PSUM->SBUF), att = t*adj with accum
    on DVE/Pool (alternating jb parity) - no CBIG offset trick needed.
  - att is pre-normalized on the (idle) GpSimd engine, removing the per-hop
    rv scaling and shrinking the combine to dif = h - az (512-wide) plus
    per-block scalar_tensor_tensor.
  - h-matmul in f32r (was plain fp32 at 4 cyc/row).
  - coarse DMA: one const blob, one xT+ndeg blob per batch, 2 adj halves
    per batch, 2 output halves per batch (~600ns SP dispatch per dma_start).
  - PSUM->SBUF copies (hST, hnat) moved to GpSimd.
"""

import sys
from contextlib import ExitStack

import numpy as np

sys.path.insert(0, "/opt/trn_rl_repo")

import concourse.bass as bass
import concourse.tile as tile
from concourse import mybir
from concourse.bass_utils import run_bass_kernel_spmd


B, N, D = 16, 1024, 128
NCORES = 8
BPC = B // NCORES        # batches per core
NB = N // 128            # 128-row blocks per matrix dim
F32 = mybir.dt.float32
F32R = mybir.dt.float32r
U8 = mybir.dt.uint8
OP = mybir.AluOpType
AF = mybir.ActivationFunctionType

# const blob column layout
C_ID, C_WW, C_WB, C_A, C_GW, C_GB = 0, 128, 256, 257, 385, 387
C_COLS = 388


def build_nc():
    nc = bass.Bass("TRN2", target_bir_lowering=False, debug=False,
                   num_devices=NCORES)

    cblob = nc.dram_tensor("cblob", [128, C_COLS], F32, kind="ExternalInput").ap()
    xTn = nc.dram_tensor("xTn", [BPC, D, N + NB], F32, kind="ExternalInput").ap()
    adjT = nc.dram_tensor("adjT", [BPC, N, N], U8, kind="ExternalInput").ap()
    out = nc.dram_tensor("out", [BPC, N, D], F32, kind="ExternalOutput").ap()

    with tile.TileContext(nc) as tc, ExitStack() as ctx:
        consts = ctx.enter_context(tc.tile_pool(name="consts", bufs=1))
        ps_a = ctx.enter_context(tc.tile_pool(name="ps_a", bufs=4, space="PSUM"))
        ps_tr = ctx.enter_context(tc.tile_pool(name="ps_tr", bufs=2, space="PSUM"))
        ps_g = ctx.enter_context(tc.tile_pool(name="ps_g", bufs=1, space="PSUM"))
        ps_ct = ctx.enter_context(tc.tile_pool(name="ps_ct", bufs=1, space="PSUM"))
        adj_pool = ctx.enter_context(tc.tile_pool(name="adj", bufs=2))
        att_pool = ctx.enter_context(tc.tile_pool(name="att", bufs=2))
        work = ctx.enter_context(tc.tile_pool(name="work", bufs=2))
        hop = ctx.enter_context(tc.tile_pool(name="hop", bufs=4))

        # ---- constants: one DMA, then on-chip prep ----------------------
        cb = consts.tile([128, C_COLS], F32, tag="cb")
        nc.sync.dma_start(cb[:, :], cblob[:, :])
        ident = cb[:, C_ID:C_ID + 128]
        wb_sb = cb[:, C_WB:C_WB + 1]
        a_sb = cb[:, C_A:C_A + 128]
        gb_sb = cb[0:1, C_GB:C_GB + 1]

        identr = consts.tile([128, 128], F32R, tag="identr")
        nc.vector.tensor_copy(identr[:, :], ident)
        wwT_sb = consts.tile([D, D], F32R, tag="wwT")
        nc.vector.tensor_copy(wwT_sb[:, :], cb[:, C_WW:C_WW + 128])
        gwr_sb = consts.tile([D, 2], F32R, tag="gwr")
        nc.vector.tensor_copy(gwr_sb[:, :], cb[:, C_GW:C_GW + 2])
        ngb_sb = consts.tile([1, 1], F32, tag="ngb")
        nc.vector.tensor_scalar(ngb_sb[:, :], gb_sb, -1.0, None, OP.mult)

        # PE warm-up on a memset tile (no DMA dependency): keeps the HAM
        # clock ramping toward 2.4GHz while the first DMAs land.
        warm_f = consts.tile([128, 512], F32, tag="warm_f")
        nc.vector.memset(warm_f[:, :], 0.0)
        warm_mv = consts.tile([128, 512], F32R, tag="warm_mv")
        nc.vector.tensor_copy(warm_mv[:, :], warm_f[:, :])
        for w in range(14):
            wp = ps_tr.tile([128, 512], F32, tag="ps_tr")
            nc.tensor.matmul(wp[:, :], warm_mv[:, 0:128], warm_mv[:, :],
                             start=True, stop=True)

        # S = A + A^T (stays for the whole kernel)
        s_sb = consts.tile([D, D], F32R, tag="smat")
        at_ps = ps_tr.tile([128, 512], F32, tag="ps_tr")
        nc.tensor.transpose(at_ps[:, 0:128], a_sb, ident)
        nc.vector.tensor_tensor(s_sb[:, :], a_sb, at_ps[:, 0:128], OP.add)

        # ---- input DMAs (few, coarse; SP dispatch is ~600ns each) -------
        xTn_sb = [None] * BPC
        adj_sb = [None] * BPC
        for b in range(BPC):
            xTn_sb[b] = work.tile([D, N + NB], F32R, tag="xTn", name="xTn_sb")
            nc.gpsimd.dma_start(xTn_sb[b][:, :], xTn[b, :, :])
            adj_sb[b] = adj_pool.tile([128, NB * N], U8, tag="adj", name="adj_sb")
            src = adjT[b, :, :].rearrange("(jb p) i -> p jb i", p=128)
            for hh in range(2):
                nc.sync.dma_start(
                    adj_sb[b][:, hh * 4 * N:(hh + 1) * 4 * N].rearrange(
                        "p (jb i) -> p jb i", jb=4),
                    src[:, hh * 4:(hh + 1) * 4, :])

        def phase_prologue(b, st):
            xT = xTn_sb[b]
            # hT[o, n] = sum_d WwT[d, o] xT[d, n] + Wb[o]
            hT_sb = work.tile([D, N], F32R, tag="hT")
            for ih in range(2):
                ph = ps_a.tile([128, 512], F32, tag="ps_a")
                nc.tensor.matmul(ph[:, :], wwT_sb[:, :],
                                 xT[:, ih * 512:(ih + 1) * 512],
                                 start=True, stop=True)
                nc.scalar.activation(hT_sb[:, ih * 512:(ih + 1) * 512], ph[:, :],
                                     AF.Identity, bias=wb_sb, scale=1.0)

            # hST[e, n] = sum_o S[o, e] hT[o, n]   (S symmetric)
            hST_sb = work.tile([D, N], F32R, tag="hST")
            for ih in range(2):
                ph = ps_a.tile([128, 512], F32, tag="ps_a")
                nc.tensor.matmul(ph[:, :], s_sb[:, :],
                                 hT_sb[:, ih * 512:(ih + 1) * 512],
                                 start=True, stop=True)
                nc.scalar.copy(hST_sb[:, ih * 512:(ih + 1) * 512], ph[:, :])

            # h in natural layout [node-in-block, nb*128 + f]
            hnat_sb = work.tile([128, N], F32R, tag="hnat")
            for half in range(2):
                pt = ps_tr.tile([128, 512], F32R, tag="ps_tr")
                for q in range(4):
                    nb = half * 4 + q
                    nc.tensor.transpose(pt[:, q * 128:(q + 1) * 128],
                                        hT_sb[:, nb * 128:(nb + 1) * 128],
                                        identr[:, :])
                nc.scalar.copy(hnat_sb[:, half * 512:(half + 1) * 512],
                               pt[:, :])
            st.update(hT=hT_sb, hST=hST_sb, hnat=hnat_sb,
                      ndeg=xT[:, N:N + NB])

        def phase_att(b, st):
            # attT[j, i] = adj[i, j] * exp(e_sym[j, i]) / denom_j
            hT_sb, hST_sb = st["hT"], st["hST"]
            adjb = adj_sb[b]
            attT_sb = att_pool.tile([128, NB * N], F32R, tag="att")
            acc_sb = work.tile([D, 2 * NB], F32, tag="acc")
            for jb in range(NB):
                for ih in range(2):
                    pe = ps_a.tile([128, 512], F32, tag="ps_a")
                    nc.tensor.matmul(pe[:, :],
                                     hST_sb[:, jb * 128:(jb + 1) * 128],
                                     hT_sb[:, ih * 512:(ih + 1) * 512],
                                     start=True, stop=True)
                    texp = work.tile([128, 512], F32, tag="texp", bufs=3)
                    nc.scalar.activation(texp[:, :], pe[:, :], AF.Exp)
                    seg = attT_sb[:, jb * N + ih * 512: jb * N + (ih + 1) * 512]
                    nc.vector.scalar_tensor_tensor(
                        seg, texp[:, :], 1.0,
                        adjb[:, jb * N + ih * 512: jb * N + (ih + 1) * 512],
                        OP.mult, OP.mult,
                        accum_out=acc_sb[:, ih * NB + jb: ih * NB + jb + 1])

            # denom = masked-exp row sums + (N - deg);  inv = 1/denom
            inv_sb = work.tile([D, NB], F32, tag="inv")
            nc.vector.tensor_tensor(inv_sb[:, :], acc_sb[:, 0:NB],
                                    acc_sb[:, NB:2 * NB], OP.add)
            nc.vector.tensor_tensor(inv_sb[:, :], inv_sb[:, :],
                                    st["ndeg"], OP.add)
            nc.vector.reciprocal(inv_sb[:, :], inv_sb[:, :])
            # rv scaled by 1/denom for the first hop's stationary operand
            rvs = hop.tile([128, N], F32R, tag="rvs")
            hnat_sb = st["hnat"]
            for nb in range(NB):
                nc.vector.tensor_scalar_mul(rvs[:, nb * 128:(nb + 1) * 128],
                                            hnat_sb[:, nb * 128:(nb + 1) * 128],
                                            inv_sb[:, nb:nb + 1])
            st.update(att=attT_sb, inv=inv_sb, rv=rvs)

        def phase_hop(b, st, k):
            last = (k == 2)
            hT_sb, hnat_sb = st["hT"], st["hnat"]
            attT_sb, rv = st["att"], st["rv"]
            # azT[f, i] = sum_j rv[j, f] attT[j, i]
            azT_sb = hop.tile([128, N], F32R, tag="azT", bufs=2)
            for ih in range(2):
                paz = ps_a.tile([128, 512], F32, tag="ps_a")
                for jb in range(NB):
                    nc.tensor.matmul(
                        paz[:, :], rv[:, jb * 128:(jb + 1) * 128],
                        attT_sb[:, jb * N + ih * 512: jb * N + (ih + 1) * 512],
                        start=(jb == 0), stop=(jb == NB - 1))
                nc.scalar.activation(azT_sb[:, ih * 512:(ih + 1) * 512],
                                     paz[:, :], AF.Relu)

            # gate: coeff = sigmoid(gw1.h + gw2.az + gb) per node, as
            # 1/(1 + exp(-pre)) to stay in the exp LUT set.
            en_sb = hop.tile([1, N], F32, tag="coeff", bufs=2)
            for ih in range(2):
                pg = ps_g.tile([1, 512], F32, tag="ps_g")
                nc.tensor.matmul(pg[:, :], gwr_sb[:, 0:1],
                                 hT_sb[:, ih * 512:(ih + 1) * 512],
                                 start=True, stop=False)
                nc.tensor.matmul(pg[:, :], gwr_sb[:, 1:2],
                                 azT_sb[:, ih * 512:(ih + 1) * 512],
                                 start=False, stop=True)
                nc.scalar.activation(en_sb[:, ih * 512:(ih + 1) * 512],
                                     pg[:, :], AF.Exp, bias=ngb_sb[:, :],
                                     scale=-1.0)

            # transpose exp(-pre) to per-partition scalars, finish the
            # sigmoid there:  c = 1/(1+e)
            ct_ps = ps_ct.tile([128, NB], F32, tag="ps_ct")
            for nb in range(NB):
                nc.tensor.transpose(ct_ps[:, nb:nb + 1],
                                    en_sb[0:1, nb * 128:(nb + 1) * 128],
                                    ident[0:1, 0:1])
            # coeff c = 1/(1+e); w1 = c (*1/denom unless last),
            # w2 = 1-c = e*c (*1/denom unless last)
            w1 = hop.tile([128, NB], F32, tag="w1", bufs=2)
            w2 = hop.tile([128, NB], F32, tag="w2", bufs=2)
            nc.vector.tensor_scalar(w1[:, :], ct_ps[:, :], 1.0, None, OP.add)
            nc.vector.reciprocal(w1[:, :], w1[:, :])
            nc.vector.tensor_tensor(w2[:, :], ct_ps[:, :], w1[:, :], OP.mult)
            if not last:
                nc.vector.tensor_tensor(w1[:, :], w1[:, :], st["inv"], OP.mult)
                nc.vector.tensor_tensor(w2[:, :], w2[:, :], st["inv"], OP.mult)

            # az back to natural layout, scale by w2, combine with h
            rv_new = hop.tile([128, N], F32 if last else F32R, tag="rvs")
            azs = hop.tile([128, N], F32, tag="azs", bufs=2)
            for half in range(2):
                pt = ps_tr.tile([128, 512], F32R, tag="ps_tr")
                for q in range(4):
                    nb = half * 4 + q
                    nc.tensor.transpose(pt[:, q * 128:(q + 1) * 128],
                                        azT_sb[:, nb * 128:(nb + 1) * 128],
                                        identr[:, :])
                for q in range(4):
                    nb = half * 4 + q
                    sl = slice(nb * 128, (nb + 1) * 128)
                    nc.scalar.activation(azs[:, sl], pt[:, q * 128:(q + 1) * 128],
                                         AF.Copy, scale=w2[:, nb:nb + 1])
                    nc.vector.scalar_tensor_tensor(
                        rv_new[:, sl], hnat_sb[:, sl], w1[:, nb:nb + 1],
                        azs[:, sl], OP.mult, OP.add)
                hsl = slice(half * 512, (half + 1) * 512)
                if last:
                    nc.sync.dma_start(
                        out[b, half * 512:(half + 1) * 512, :].rearrange(
                            "(q p) f -> p q f", p=128),
                        rv_new[:, hsl].rearrange("p (q f) -> p q f", f=128))
            if not last:
                st["rv"] = rv_new

        # phase-interleaved emission: engines run ahead independently, so
        # att(b1)'s ACT/DVE/Pool pipeline overlaps hop0(b0)'s PE burst.
        states = [{} for _ in range(BPC)]
        for b in range(BPC):
            phase_prologue(b, states[b])
        for b in range(BPC):
            phase_att(b, states[b])
        for k in range(3):
            for b in range(BPC):
                phase_hop(b, states[b], k)

        # Spare per-engine nops: relocated by _fixup_waits to carry sync
        # waits that walrus cannot fit on compute-instruction structs.
        nop_insts = []
        for eng in (nc.tensor, nc.vector, nc.scalar, nc.gpsimd, nc.sync):
            for _ in range(96):
                nop_insts.append(eng.nop(nofuse=True).ins)

    _fixup_waits(nc, nop_insts)
    return nc


_FIXUP_SKIP = {"InstNoOp"}


def _fixup_waits(nc, nop_insts):
    """walrus (enable-ldw-opt=false) rejects compute instructions with more
    than one sync wait (single wait slot in the S3 structs).  Hoist
    all-but-one wait of each such instruction onto spare same-engine nop
    instructions inserted immediately before it in program order."""
    nop_set = set(id(x) for x in nop_insts)
    free_nops = {}
    for x in nop_insts:
        free_nops.setdefault(x.engine, []).append(x)
    f = nc.m.functions[0]
    for blk in f.blocks:
        insts = blk.instructions
        for i in range(len(insts) - 1, -1, -1):
            if id(insts[i]) in nop_set:
                insts.pop(i)
        i = 0
        while i < len(insts):
            inst = insts[i]
            if inst.__class__.__name__ not in _FIXUP_SKIP:
                si = inst.sync_info
                if si is not None and si.on_wait and len(si.on_wait) > 1:
                    waits = list(si.on_wait)
                    extra, keep = waits[:-1], waits[-1:]
                    inst.sync_info = mybir.SyncInfo(
                        on_wait=keep, on_update=list(si.on_update or []))
                    pool = free_nops.get(inst.engine)
                    for k, w in enumerate(extra):
                        if not pool:
                            raise RuntimeError(
                                f"out of spare nops for {inst.engine}")
                        nop = pool.pop()
                        nop.sync_info = mybir.SyncInfo(on_wait=[w], on_update=[])
                        insts.insert(i + k, nop)
                    i += len(extra)
            i += 1


_NC_CACHE = None


def _get_nc():
    global _NC_CACHE
    if _NC_CACHE is None:
        _NC_CACHE = build_nc()
    return _NC_CACHE


def _prep_in_maps(inputs):
    x = np.ascontiguousarray(np.asarray(inputs["x"], dtype=np.float32))
    adj = np.ascontiguousarray(np.asarray(inputs["adj"], dtype=np.float32))
    W_w = np.asarray(inputs["W_w"], dtype=np.float32)
    W_b = np.asarray(inputs["W_b"], dtype=np.float32)
    A = np.asarray(inputs["A"], dtype=np.float32)
    gate_w = np.asarray(inputs["gate_w"], dtype=np.float32)
    gate_b = np.asarray(inputs["gate_b"], dtype=np.float32)

    cblob = np.zeros((128, C_COLS), dtype=np.float32)
    cblob[:, C_ID:C_ID + 128] = np.eye(128, dtype=np.float32)
    cblob[:, C_WW:C_WW + 128] = W_w.T
    cblob[:, C_WB] = W_b
    cblob[:, C_A:C_A + 128] = A
    cblob[:, C_GW:C_GW + 2] = gate_w.reshape(2, D).T
    cblob[0, C_GB] = gate_b.reshape(())

    in_maps = []
    for c in range(NCORES):
        sl = slice(c * BPC, (c + 1) * BPC)
        adj_c = adj[sl]
        adjT_c = np.ascontiguousarray(
            adj_c.transpose(0, 2, 1).astype(np.uint8))
        xT_c = x[sl].transpose(0, 2, 1)                            # [BPC, D, N]
        ndeg = (N - adj_c.sum(axis=1)).astype(np.float32)          # [BPC, N]
        ndegT = ndeg.reshape(BPC, NB, 128).transpose(0, 2, 1)      # [BPC, 128, NB]
        xTn_c = np.ascontiguousarray(
            np.concatenate([xT_c, ndegT], axis=2))                 # [BPC, D, N+NB]
        in_maps.append({
            "cblob": cblob, "xTn": xTn_c, "adjT": adjT_c,
        })
    return in_maps


def _run(inputs, trace=False, **kwargs):
    nc = _get_nc()
    in_maps = _prep_in_maps(inputs)
    res = run_bass_kernel_spmd(nc, in_maps, core_ids=list(range(NCORES)),
                               trace=trace, **kwargs)
    out = np.concatenate([res.results[c]["out"] for c in range(NCORES)], axis=0)
    return out.astype(np.float32), res


def kernel(**inputs) -> np.ndarray:
    out, _ = _run(inputs, trace=False)
    return out
